# revision 31
# baseline (speedup 1.0000x reference)
"""Trainium2 Bass kernel for CapsNet DigitCaps dynamic routing (nn_DigitCaps).

Reference computation:
    u_hat[b,r,j,o] = W[r,j,o,:] @ x[b,r,:]        B,R,J,O,I = 512,1152,10,16,8
    b_ij = 0; 3 routing iterations:
        c = softmax(b_ij, axis=0)                  # over routes r, per j
        s[b,j,o] = sum_r c[r,j] * u_hat[b,r,j,o]
        v = squash(s) = s*|s|/(1+s^2)              # elementwise
        b_ij += mean_b sum_o u_hat[b,r,j,o]*v[b,j,o]
    return v[..., None]

Kernel strategy (data-parallel over batch, 8 cores, 64 rows each; u_hat is
never materialized; everything in bf16 except PSUM accumulations):
    s    = X @ (e ⊙ W2)          X: [64, 9216]=[b,(rb,i,p)], W2: [9216,160]
    M    = X^T @ V               per-(rb,i) chunk [128,160] in PSUM
    bupd[r,j] = sum_{i,o} W2 ⊙ M: ACT evacuates M to bf16 SBUF (6 of 8
                groups; DVE reads PSUM directly for the rest), DVE product
                at 2x, then a TT halving tree over (i,o) (tensor_reduce has
                no fast DVE mode; TT pairs run at 2x).
    bupd is AllGather'd (bf16, [128,90] per core) and summed locally;
    softmax is deferred: unnormalized e=exp(b) scales W2, 1/Z applied to s.
Row order is (rb, i, p) with r = rb*128+p, so the e-broadcast needs no
replication matmuls and the (i,o) reduction needs no PE ones-trick.
A junk-matmul chain anchored on b_upd keeps the PE HAM clock warm across
the tree+exchange idle window (HW-only effect; free in the cost model).
"""
import numpy as np
from contextlib import ExitStack

import ml_dtypes

import concourse.bacc as bacc
import concourse.bass as bass
import concourse.tile as tile
from concourse import mybir
from concourse.bass_utils import run_bass_kernel_spmd

F32 = mybir.dt.float32
BF16 = mybir.dt.bfloat16

B, R, J, O, I = 512, 1152, 10, 16, 8
N_CORES = 8
BL = B // N_CORES          # 64 batch rows per core
RI = R * I                 # 9216
NJO = J * O                # 160
NRB = R // 128             # 9 r-blocks
NCH = NRB * I              # 72 contraction chunks; chunk c = rb*8 + i
NUM_ITER = 3
PBANK = 512                # fp32 elems per PSUM bank


def emit_algorithm(nc, tc, ctx, tensors, pools, out_d, flags=()):
    (xT_s, x_s, W2_s, ONES_s, RONES_s) = tensors
    (sp, vq, wc_pool, msb_pool, p_pool, dram_pool, ps_s, ps_m, ps_tiny) = pools

    b_state = None
    for it in range(NUM_ITER):
        last = it == NUM_ITER - 1
        use_scale = it > 0

        zinv_b = None
        ebc = None
        if use_scale:
            # ebc[p,(rb,j,o)] = exp(b)[p,(rb,j)] broadcast over o  (ACT),
            # split per super-group so wc can start on sg0 early
            ebc = sp.tile([128, NRB * J * O], BF16, tag="ebc")
            for sg in range(3):
                ob = ebc[:, sg * 3 * J * O:(sg + 1) * 3 * J * O].rearrange(
                    "p (rb j o) -> p rb j o", j=J, o=O)
                ib = b_state[:, sg * 3 * J:(sg + 1) * 3 * J].rearrange(
                    "p (rb j o) -> p rb j o", j=J, o=1)
                ob2, ib2 = bass.broadcast_tensor_aps(ob, ib)
                nc.scalar.activation(ob2, ib2,
                                     mybir.ActivationFunctionType.Exp)

        # ---- weight scaling (3 super-groups of 3 rb) + s matmul ----
        s_ps = ps_s.tile([BL, NJO], F32, tag="s")
        wc = None
        if use_scale:
            wc = wc_pool.tile([128, NCH * NJO], BF16, tag="wc")
        for sg in range(3):
            if use_scale:
                lo, hi = sg * 3 * I * NJO, (sg + 1) * 3 * I * NJO
                i0 = W2_s[:, lo:hi].rearrange(
                    "p (rb i jo) -> p rb i jo", i=I, jo=NJO)
                i1 = ebc[:, sg * 3 * NJO:(sg + 1) * 3 * NJO].rearrange(
                    "p (rb i jo) -> p rb i jo", i=1, jo=NJO)
                o0 = wc[:, lo:hi].rearrange(
                    "p (rb i jo) -> p rb i jo", i=I, jo=NJO)
                i0b, i1b = bass.broadcast_tensor_aps(i0, i1)
                nc.vector.tensor_tensor(o0, i0b, i1b, op=mybir.AluOpType.mult)
            for k in range(sg * 24, (sg + 1) * 24):
                rhs = (wc if use_scale else W2_s)[:, k * NJO:(k + 1) * NJO]
                nc.tensor.matmul(s_ps[:], xT_s[:, k * BL:(k + 1) * BL], rhs,
                                 start=(k == 0), stop=(k == NCH - 1))

        if use_scale:
            # Z_j = sum_r e[r,j]; 1/Z broadcast over the 64 batch rows.
            # Emitted after wc so the DVE starts wc right when ebc lands;
            # zinv_b is only needed at squash time.
            sums_ps = ps_tiny.tile([1, NRB * J], F32, tag="tiny")
            nc.tensor.matmul(
                sums_ps[:],
                ONES_s[:, 0:1],
                ebc[:].rearrange("p (rb j o) -> p (rb j) o", j=J, o=O)[:, :, 0],
                start=True, stop=True)
            zsum = vq.tile([1, J], F32, tag="zsum")
            nc.vector.tensor_reduce(
                zsum[:],
                sums_ps[:].rearrange("p (rb j) -> p j rb", j=J),
                axis=mybir.AxisListType.X, op=mybir.AluOpType.add)
            zinv = vq.tile([1, J], F32, tag="zinv")
            nc.vector.reciprocal(zinv[:], zsum[:])
            zb_ps = ps_tiny.tile([BL, J], F32, tag="tiny")
            nc.tensor.matmul(zb_ps[:], RONES_s[:, 0:BL], zinv[:],
                             start=True, stop=True)
            zinv_b = vq.tile([BL, J], F32, tag="zinv_b")
            nc.vector.tensor_copy(zinv_b[:], zb_ps[:])

        # ---- squash (deferred softmax normalization for it > 0) ----
        src = vq.tile([BL, NJO], F32, tag="s_n")
        if use_scale:
            i0 = s_ps[:].rearrange("p (j o) -> p j o", o=O)
            i1 = zinv_b[:].rearrange("p (j o) -> p j o", o=1)
            i0b, i1b = bass.broadcast_tensor_aps(i0, i1)
            nc.vector.tensor_tensor(
                src[:].rearrange("p (j o) -> p j o", o=O), i0b, i1b,
                op=mybir.AluOpType.mult)
        else:
            nc.vector.tensor_scalar_mul(src[:], s_ps[:], 1.0 / R)
        sq = vq.tile([BL, NJO], F32, tag="sq")
        nc.scalar.activation(sq[:], src[:], mybir.ActivationFunctionType.Square)
        sab = vq.tile([BL, NJO], F32, tag="sab")
        nc.scalar.activation(sab[:], src[:], mybir.ActivationFunctionType.Abs)
        nc.vector.tensor_scalar_add(sq[:], sq[:], 1.0)
        rec = vq.tile([BL, NJO], F32, tag="rec")
        nc.vector.reciprocal(rec[:], sq[:])
        num = vq.tile([BL, NJO], F32, tag="num")
        nc.vector.tensor_mul(num[:], src[:], sab[:])

        if last:
            vout = vq.tile([BL, NJO], F32, tag="vout")
            nc.vector.tensor_mul(vout[:], num[:], rec[:])
            nc.sync.dma_start(out_d[:], vout[:])
            break

        # vpad = v * (1/B)  (folds the agreement batch-mean scale), bf16
        vpad = vq.tile([BL, NJO], BF16, tag="vpad")
        nc.vector.scalar_tensor_tensor(vpad[:], num[:], 1.0 / B, rec[:],
                                       op0=mybir.AluOpType.mult,
                                       op1=mybir.AluOpType.mult)

        # ---- M matmul + W2 contraction -> b_upd ----
        # Per rb-group: 8 chunk matmuls into a 3-bank PSUM tile (3,3,2 chunks
        # per bank), ACT copies to bf16 SBUF; DVE products per 3-rb super-
        # group into one shared P tile, then a TT halving tree reduces (i,o)
        # (tensor_reduce has no fast DVE mode, TT pairs run at 2x).
        # 8 groups of 9 chunks; within a 3-bank PSUM tile chunks sit at a
        # uniform 176-elem stride (3 per bank: 176*2+160 = 512) so one ACT
        # copy (or one direct DVE product) covers the whole group.
        P = p_pool.tile([128, NCH * NJO], BF16, tag="P")
        t1 = sp.tile([128, NCH * NJO // 2], BF16, tag="t1", bufs=1)
        t2 = sp.tile([128, NCH * NJO // 4], BF16, tag="t2", bufs=1)
        t3 = sp.tile([128, NRB * NJO], BF16, tag="t3", bufs=1)
        CST = 176
        rb_emitted = 0
        for g in range(8):
            mps = ps_m.tile([128, 3 * PBANK], F32, tag="m")
            for q in range(9):
                c = g * 9 + q
                off = (q // 3) * PBANK + (q % 3) * CST
                nc.tensor.matmul(mps[:, off:off + NJO],
                                 x_s[:, c * 128:(c + 1) * 128], vpad[:],
                                 start=True, stop=True)
            lo = g * 9 * NJO
            _a = mps[:]
            mv = bass.AP(_a.tensor, _a.offset,
                         [list(_a.ap[0]), [PBANK, 3], [CST, 3], [1, NJO]])
            # ACT evacuates PSUM to bf16, DVE multiplies at 2x
            m_sb = msb_pool.tile([128, 9 * NJO], BF16, tag="m_sb",
                                 name=f"m_sb{it}_{g}")
            nc.scalar.activation(
                m_sb[:].rearrange("p (bk c jo) -> p bk c jo",
                                  bk=3, jo=NJO),
                mv, mybir.ActivationFunctionType.Copy)
            nc.vector.tensor_mul(P[:, lo:lo + 9 * NJO],
                                 W2_s[:, lo:lo + 9 * NJO], m_sb[:])
            if g % 2 == 1:
                # tree level 1 for rb-blocks fully covered so far
                rb_done = (9 * g + 9) // I
                rb_prev = rb_emitted
                rb_emitted = rb_done
                nrb = rb_done - rb_prev
                if nrb > 0:
                    plo = rb_prev * I * NJO
                    pv = P[:, plo:plo + nrb * I * NJO].rearrange(
                        "p (gg i jo) -> p gg i jo", i=I, jo=NJO)
                    with nc.allow_low_precision(reason="b values ~1e-1"):
                        nc.vector.tensor_add(
                            t1[:, plo // 2:plo // 2 + nrb * I * NJO // 2]
                            .rearrange("p (gg i jo) -> p gg i jo",
                                       i=4, jo=NJO),
                            pv[:, :, 0:4], pv[:, :, 4:8])
                # levels 2+3 for the first 4 rb-blocks fill product gaps
                if g == 3:
                    t1a = t1[:, 0:4 * 4 * NJO].rearrange(
                        "p (gg i jo) -> p gg i jo", i=4, jo=NJO)
                    with nc.allow_low_precision(reason="b values ~1e-1"):
                        nc.vector.tensor_add(
                            t2[:, 0:4 * 2 * NJO].rearrange(
                                "p (gg i jo) -> p gg i jo", i=2, jo=NJO),
                            t1a[:, :, 0:2], t1a[:, :, 2:4])
                        t2a = t2[:, 0:4 * 2 * NJO].rearrange(
                            "p (gg i jo) -> p gg i jo", i=2, jo=NJO)
                        nc.vector.tensor_add(
                            t3[:, 0:4 * NJO].rearrange(
                                "p (gg i jo) -> p gg i jo", i=1, jo=NJO),
                            t2a[:, :, 0:1], t2a[:, :, 1:2])
        # rest of the halving tree: i (4 -> 1) for rbs 4-8, then o (16 -> 1)
        b_upd = sp.tile([128, NRB * J], BF16, tag="b_upd")
        with nc.allow_low_precision(reason="b values ~1e-1, bf16 ok"):
            t1v = t1[:].rearrange("p (g i jo) -> p g i jo", i=4, jo=NJO)
            nc.vector.tensor_add(
                t2[:, 4 * 2 * NJO:].rearrange("p (g i jo) -> p g i jo",
                                              i=2, jo=NJO),
                t1v[:, 4:, 0:2], t1v[:, 4:, 2:4])
            t2v = t2[:].rearrange("p (g i jo) -> p g i jo", i=2, jo=NJO)
            nc.vector.tensor_add(
                t3[:, 4 * NJO:].rearrange("p (g i jo) -> p g i jo",
                                          i=1, jo=NJO),
                t2v[:, 4:, 0:1], t2v[:, 4:, 1:2])
            cur = t3
            no = O
            while no > 1:
                cv = cur[:].rearrange("p (gj o) -> p gj o", o=no)
                if no > 2:
                    nxt = sp.tile([128, NRB * J * no // 2], BF16,
                                  tag=f"to{no}", name=f"to{it}_{no}")
                else:
                    nxt = b_upd
                nc.vector.tensor_add(
                    nxt[:].rearrange("p (gj o) -> p gj o", o=no // 2),
                    cv[:, :, 0:no // 2], cv[:, :, no // 2:no])
                cur = nxt
                no //= 2

        # ---- cross-core sum of b_upd (AllGather + local sum, bf16) ----
        cc_in = dram_pool.tile([128, NRB * J], BF16, tag="cc_in")
        nc.sync.dma_start(cc_in[:], b_upd[:])
        cc_out = dram_pool.tile([N_CORES * 128, NRB * J], BF16, tag="cc_outg")
        if "noexch" in flags:
            # timing-only variant: skip the collective (numerics wrong)
            for q in range(N_CORES):
                nc.sync.dma_start(
                    cc_out[:].rearrange("(k p) f -> k p f", p=128)[q], cc_in[:])
        else:
            nc.gpsimd.collective_compute(
                "AllGather", mybir.AluOpType.bypass,
                replica_groups=[list(range(N_CORES))],
                ins=[cc_in.opt()], outs=[cc_out.opt()])
        # keep the PE's HAM clock warm across the tree+exchange gap: a junk
        # matmul chain into the (currently dead) s_ps bank, anchored on b_upd
        # so it fills the idle window without delaying next-iteration work
        nc.tensor.matmul(s_ps[:, 0:NRB * J], xT_s[:, 0:BL], b_upd[:],
                         start=True, stop=True)
        for w in range(54):
            nc.tensor.matmul(s_ps[:], xT_s[:, 0:BL],
                             W2_s[:, (w % 16) * NJO:(w % 16 + 1) * NJO],
                             start=True, stop=True)

        # fold the old b_state into the gather sum as a 9th slot
        nslot = N_CORES + (1 if b_state is not None else 0)
        gath = sp.tile([128, (N_CORES + 1) * NRB * J], BF16, tag="gath")
        if b_state is not None:
            nc.vector.tensor_copy(
                gath[:, N_CORES * NRB * J:(N_CORES + 1) * NRB * J], b_state[:])
        nc.sync.dma_start(
            gath[:, 0:N_CORES * NRB * J].rearrange("p (k f) -> p k f",
                                                   f=NRB * J),
            cc_out[:].rearrange("(k p) f -> p k f", p=128))
        upd = sp.tile([128, NRB * J], BF16, tag=f"bstate{it}")
        with nc.allow_low_precision(reason="b values ~1e-1, bf16 ok"):
            for sg in range(3):
                nc.vector.tensor_reduce(
                    upd[:, sg * 3 * J:(sg + 1) * 3 * J],
                    gath[:, 0:nslot * NRB * J]
                    .rearrange("p (k f) -> p f k", f=NRB * J)
                    [:, sg * 3 * J:(sg + 1) * 3 * J, :],
                    axis=mybir.AxisListType.X, op=mybir.AluOpType.add)
        b_state = upd


def build_nc(reps=1, flags=()):
    nc = bacc.Bacc("TRN2", target_bir_lowering=False, debug=False,
                   num_devices=N_CORES)
    xT_d = nc.dram_tensor("xT", [RI, BL], BF16, kind="ExternalInput")
    x_d = nc.dram_tensor("x", [BL, RI], BF16, kind="ExternalInput")
    W2_d = nc.dram_tensor("W2", [RI, NJO], BF16, kind="ExternalInput")
    ONES_d = nc.dram_tensor("ONES", [128, 1], BF16, kind="ExternalInput")
    RONES_d = nc.dram_tensor("RONES", [1, 128], F32, kind="ExternalInput")
    out_d = nc.dram_tensor("out", [BL, NJO], F32, kind="ExternalOutput")

    with tile.TileContext(nc) as tc:
        with ExitStack() as ctx:
            pers = ctx.enter_context(tc.tile_pool(name="pers", bufs=1))
            sp = ctx.enter_context(tc.tile_pool(name="sp", bufs=2))
            vq = ctx.enter_context(tc.tile_pool(name="vq", bufs=2))
            wc_pool = ctx.enter_context(tc.tile_pool(name="wcp", bufs=1))
            msb_pool = ctx.enter_context(tc.tile_pool(name="msb", bufs=2))
            p_pool = ctx.enter_context(tc.tile_pool(name="pp", bufs=1))
            dram_pool = ctx.enter_context(
                tc.tile_pool(name="dram", bufs=2, space="DRAM"))
            ps_s = ctx.enter_context(tc.tile_pool(name="ps_s", bufs=1, space="PSUM"))
            ps_m = ctx.enter_context(tc.tile_pool(name="ps_m", bufs=2, space="PSUM"))
            ps_tiny = ctx.enter_context(tc.tile_pool(name="ps_y", bufs=1, space="PSUM"))

            xT_s = pers.tile([128, NCH * BL], BF16)
            x_s = pers.tile([BL, RI], BF16)
            W2_s = pers.tile([128, NCH * NJO], BF16)
            ONES_s = pers.tile([128, 1], BF16)
            RONES_s = pers.tile([1, 128], F32)

            # input loads: W2 on the scalar queue, xT on sync, x on gpsimd
            for g in range(NRB):
                nc.scalar.dma_start(
                    W2_s[:, g * I * NJO:(g + 1) * I * NJO]
                    .rearrange("p (c n) -> p c n", n=NJO),
                    W2_d[:].rearrange("(c p) n -> p c n", p=128)
                    [:, g * I:(g + 1) * I, :])
                nc.sync.dma_start(
                    xT_s[:, g * I * BL:(g + 1) * I * BL]
                    .rearrange("p (c m) -> p c m", m=BL),
                    xT_d[:].rearrange("(c p) m -> p c m", p=128)
                    [:, g * I:(g + 1) * I, :])
            for g in range(4):
                nc.gpsimd.dma_start(x_s[:, g * 2304:(g + 1) * 2304],
                                    x_d[:, g * 2304:(g + 1) * 2304])
            nc.sync.dma_start(ONES_s[:], ONES_d[:])
            nc.sync.dma_start(RONES_s[:], RONES_d[:])

            tensors = (xT_s, x_s, W2_s, ONES_s, RONES_s)
            pools = (sp, vq, wc_pool, msb_pool, p_pool, dram_pool,
                     ps_s, ps_m, ps_tiny)
            for rep in range(reps):
                emit_algorithm(nc, tc, ctx, tensors, pools, out_d, flags)

    nc.compile()
    return nc


def make_host_inputs(x, W):
    """Per-core in_maps; row order (rb, i, p) with r = rb*128 + p."""
    x = np.asarray(x, dtype=np.float32)
    W = np.asarray(W, dtype=np.float32)
    bf = ml_dtypes.bfloat16

    # W2h[rb*1024 + i*128 + p, j*16+o] = W[rb*128+p, j, o, i]
    Wt = W.transpose(0, 3, 1, 2).reshape(R, I, NJO)          # [r, i, jo]
    W2h = np.ascontiguousarray(
        Wt.reshape(NRB, 128, I, NJO).transpose(0, 2, 1, 3)
        .reshape(RI, NJO).astype(bf))

    ONES = np.ones((128, 1), bf)
    RONES = np.ones((1, 128), np.float32)

    in_maps = []
    for c in range(N_CORES):
        xc = x[c * BL:(c + 1) * BL]                          # [64, R, I]
        xh = np.ascontiguousarray(
            xc.reshape(BL, NRB, 128, I).transpose(0, 1, 3, 2)
            .reshape(BL, RI).astype(bf))
        in_maps.append({
            "x": xh,
            "xT": np.ascontiguousarray(xh.T),
            "W2": W2h,
            "ONES": ONES,
            "RONES": RONES,
        })
    return in_maps


def assemble_output(results):
    return np.concatenate(
        [results[c]["out"].reshape(BL, J, O, 1) for c in range(N_CORES)],
        axis=0).astype(np.float32)


_NC_CACHE = {}


def kernel(x, W):
    if "nc" not in _NC_CACHE:
        _NC_CACHE["nc"] = build_nc(reps=1)
    nc = _NC_CACHE["nc"]
    in_maps = make_host_inputs(x, W)
    res = run_bass_kernel_spmd(nc, in_maps, list(range(N_CORES)))
    return assemble_output(res.results)


if __name__ == "__main__":
    import reference
    inputs = reference.setup_inputs()
    expected = np.asarray(reference.reference(**inputs))
    got = kernel(np.asarray(inputs["x"]), np.asarray(inputs["W"]))
    err = np.abs(got - expected).max()
    rel = err / np.abs(expected).max()
    print("abs err:", err, "scale-rel err:", rel)
